# revision 1
# baseline (speedup 1.0000x reference)
"""DiffuserSelfAttention (sparse attention) Trainium2 Bass kernel.

Strategy: the edge-list graph attention is reformulated as dense masked
attention (density ~35%), head-parallel across the 8 NeuronCores (NH=8
heads, one head per core, zero collectives).

Per core (head h):
  1. qkT = [Wq_h/8 | Wk_h] @ hsT  (biases folded in via a ones-row)
  2. v [1024,64] (normal layout, i on partitions)
  3. St[j,i] = sum_d kT[d,j] qT[d,i]     (PE, K=64)
  4. Wt = exp(St) * adjmask              (ScalarE exp + VectorE mul)
  5. 5 rounds: h <- 0.9 * (Wt^T h)/denom + 0.1 v ; denom comes from a
     ones column appended to h in round 0 (exact softmax denominator).
     Round 0 is interleaved with the score/exp pipeline per j-tile.

A PE warmup burst of dummy matmuls runs during the input-DMA window so
the HAM clock gate reaches 2.4 GHz before real work starts.

All matmuls in bf16 (measured end-to-end rel err ~2.4e-3 vs f32 ref).

Self-contained: hardcodes B=1, S=1024, HIDDEN=512, NH=8, HD=64.
"""

import numpy as np
import ml_dtypes

S = 1024
HIDDEN = 512
NH = 8
HD = 64
P = 128
NT_S = S // P            # 8 node tiles
KDIM = HIDDEN            # contraction dim (biases are zero per the spec)
NT_K = KDIM // P         # 4 contraction tiles for projections
ALPHA = 0.1
N_ROUNDS = 5
WARMUP_MMS = 6          # dummy matmuls to warm the PE clock gate during DMA

_CACHED = {}


def _build_module():
    import concourse.bass as bass
    import concourse.tile as tile
    from concourse import bacc
    import concourse.mybir as mybir

    f32 = mybir.dt.float32
    bf16 = mybir.dt.bfloat16
    AF = mybir.ActivationFunctionType
    ts = bass.ts

    nc = bacc.Bacc("TRN2", target_bir_lowering=False, debug=False, num_devices=NH)

    hsT_d = nc.dram_tensor("hsT", [KDIM, S], bf16, kind="ExternalInput")
    wqkv_d = nc.dram_tensor("wqkv", [P, NT_K * (P + HD)], bf16, kind="ExternalInput")
    adjT_d = nc.dram_tensor("adjT", [S, S], bf16, kind="ExternalInput")
    out_d = nc.dram_tensor("out", [S, HD], f32, kind="ExternalOutput")

    hsT_t = hsT_d.ap().rearrange("(ko p) i -> p ko i", p=P)
    adjT_t = adjT_d.ap().rearrange("(t p) i -> p t i", p=P)
    out_t = out_d.ap().rearrange("(t p) d -> p t d", p=P)

    with tile.TileContext(nc) as tc:
        with (
            tc.tile_pool(name="singles", bufs=1) as singles,
            tc.tile_pool(name="work", bufs=3) as work,
            tc.tile_pool(name="psum_big", bufs=3, space="PSUM") as psum_big,
            tc.tile_pool(name="psum_small", bufs=2, space="PSUM") as psum_small,
        ):
            # ---- PE warmup: dummy matmuls on scratch while inputs DMA in ----
            scratch = singles.tile([P, 512], bf16)
            nc.gpsimd.memset(scratch[:], 0.0)
            ps_w = psum_small.tile([P, 512], f32, name="ps_w", tag="ps_small")
            for _ in range(WARMUP_MMS):
                nc.tensor.matmul(
                    ps_w[:], scratch[:, :P], scratch[:], start=True, stop=True
                )

            # ---- load inputs (small weights first, then hsT, then adjT) ----
            wqkv_sb = singles.tile([P, NT_K, P + HD], bf16)
            # host pre-packs per-partition-contiguous lines (2KB-class DMA
            # lines instead of 384B ones — small lines pay ~4x descriptor
            # overhead and gated the first matmul)
            nc.sync.dma_start(
                wqkv_sb[:], wqkv_d.ap().rearrange("p (ko m) -> p ko m", ko=NT_K)
            )
            hsT_sb = singles.tile([P, NT_K, S], bf16)
            for ke in range(NT_K):
                nc.sync.dma_start(hsT_sb[:, ke, :], hsT_t[:, ke, :])
            adjT_sb = singles.tile([P, NT_S, S], bf16)
            for jc in range(4):
                nc.sync.dma_start(
                    adjT_sb[:, 2 * jc : 2 * jc + 2, :], adjT_t[:, 2 * jc : 2 * jc + 2, :]
                )

            # ---- persistent intermediates ----
            qT_sb = singles.tile([HD, S], bf16)
            kT_sb = singles.tile([HD, S], bf16)
            wt_sb = singles.tile([P, NT_S, S], bf16)       # masked exp(score), [j, i]
            h0_sb = singles.tile([P, NT_S, HD + 1], bf16)  # v with ones column
            av_sb = singles.tile([P, NT_S, HD], f32)       # 0.1 * v
            h_a = singles.tile([P, NT_S, HD], bf16)
            h_b = singles.tile([P, NT_S, HD], bf16)
            recip_sb = singles.tile([P, NT_S, 1], f32)     # 0.9 / denom per i
            out_sb = singles.tile([P, NT_S, HD], f32)

            # ---- stages 1+2: v / q / k projections, ke-major so matmuls
            # start as each hsT k-tile lands from DMA.
            # Accumulation-group rule (verified on HW): start=True clears
            # group state BANK-wide, so within a shared bank only the very
            # first matmul carries start=True; bank-disjoint groups keep
            # normal start flags.
            nc.vector.memset(h0_sb[:, :, HD : HD + 1], 1.0)
            ps_vs = [
                psum_small.tile([P, 4, HD], f32, name=f"ps_v{ih}", tag="ps_small")
                for ih in range(2)
            ]
            ps_q = psum_big.tile([HD, S], f32, name="ps_q", tag="ps_big")
            ps_k = psum_big.tile([HD, S], f32, name="ps_k", tag="ps_big")
            # qk first (DMA-paced over hsT k-tiles); the qT/kT copies then
            # overlap the v matmuls so PE streams into the score stage with
            # no idle gap (a >1us gap re-throttles the PE clock).
            for ke in range(NT_K):
                for w0, ps in ((0, ps_q), (HD, ps_k)):
                    for n in range(2):
                        nc.tensor.matmul(
                            ps[:, ts(n, 512)],
                            wqkv_sb[:, ke, w0 : w0 + HD],
                            hsT_sb[:, ke, ts(n, 512)],
                            start=(ke == 0),
                            stop=(ke == NT_K - 1),
                            skip_group_check=True,
                        )
            for n in range(2):
                nc.scalar.activation(
                    out=qT_sb[:, ts(n, 512)], in_=ps_q[:, ts(n, 512)], func=AF.Copy
                )
                nc.scalar.activation(
                    out=kT_sb[:, ts(n, 512)], in_=ps_k[:, ts(n, 512)], func=AF.Copy
                )
            for ke in range(NT_K):
                for it in range(NT_S):
                    nc.tensor.matmul(
                        ps_vs[it // 4][:, it % 4, :],
                        hsT_sb[:, ke, ts(it, P)],
                        wqkv_sb[:, ke, P : P + HD],
                        start=(ke == 0 and it % 4 == 0),
                        stop=(ke == NT_K - 1 and it % 4 == 3),
                        skip_group_check=True,
                    )
            for ih in range(2):
                sl = slice(ih * 4, ih * 4 + 4)
                nc.vector.tensor_copy(out=h0_sb[:, sl, :HD], in_=ps_vs[ih][:])
                nc.vector.tensor_scalar_mul(av_sb[:, sl, :], ps_vs[ih][:], ALPHA)

            # ---- stage 3+round0: per j-tile score -> exp -> mask -> accumulate ----
            ps_p0 = []
            for ih in range(2):
                ps = psum_small.tile(
                    [P, 4, HD + 1], f32, name=f"ps_p0_{ih}", tag="ps_small"
                )
                ps_p0.append(ps)
            # Software pipeline: the round-0 matmuls for half (jt, n) are
            # emitted one half-step later, so PE streams score matmuls while
            # ACT/DVE run exp+mask on the previous half — PE never waits on
            # the in-flight chain and the HAM clock gate stays warm.
            def emit_r0(jt, n):
                # Groups interleave within each PSUM bank, so only the bank's
                # FIRST matmul carries start=True (start clears accumulation
                # state bank-wide — verified on HW).
                for il in range(4):
                    it = n * 4 + il
                    nc.tensor.matmul(
                        ps_p0[n][:, il, :],
                        wt_sb[:, jt, ts(it, P)],
                        h0_sb[:, jt, :],
                        start=(jt == 0 and il == 0),
                        stop=(jt == NT_S - 1 and il == 3),
                        skip_group_check=True,
                    )

            lagged = []
            for jt in range(NT_S):
                ps_s = psum_big.tile([P, S], f32, name=f"ps_s{jt}", tag="ps_big")
                for n in range(2):
                    nc.tensor.matmul(
                        ps_s[:, ts(n, 512)],
                        kT_sb[:, ts(jt, P)],
                        qT_sb[:, ts(n, 512)],
                        start=True,
                        stop=True,
                    )
                # last two j-tiles at half-width so their round-0 matmuls
                # start per-half, shortening the pipeline drain before round 1
                if jt < NT_S - 2:
                    nc.scalar.activation(
                        out=wt_sb[:, jt, :], in_=ps_s[:], func=AF.Exp
                    )
                    nc.vector.tensor_mul(
                        out=wt_sb[:, jt, :],
                        in0=wt_sb[:, jt, :],
                        in1=adjT_sb[:, jt, :],
                    )
                    for n in range(2):
                        lagged.append((jt, n))
                        if len(lagged) > 2:
                            emit_r0(*lagged.pop(0))
                else:
                    for n in range(2):
                        nc.scalar.activation(
                            out=wt_sb[:, jt, ts(n, 512)],
                            in_=ps_s[:, ts(n, 512)],
                            func=AF.Exp,
                        )
                        nc.vector.tensor_mul(
                            out=wt_sb[:, jt, ts(n, 512)],
                            in0=wt_sb[:, jt, ts(n, 512)],
                            in1=adjT_sb[:, jt, ts(n, 512)],
                        )
                        lagged.append((jt, n))
                        if len(lagged) > 2:
                            emit_r0(*lagged.pop(0))
            while lagged:
                emit_r0(*lagged.pop(0))


            def finish_round(ps, sl, r, dst):
                """normalize + residual for a chunk of i-tiles"""
                wid = sl.stop - sl.start
                if r == 0:
                    den_t = work.tile([P, wid, 1], f32, name="den_t", tag="den_t")
                    nc.vector.tensor_scalar_mul(
                        den_t[:], ps[:, :, HD : HD + 1], 1.0 / (1.0 - ALPHA)
                    )
                    nc.vector.reciprocal(recip_sb[:, sl, :], den_t[:])
                tmp = work.tile([P, wid, HD], f32, name="tmp_sc", tag="tmp_sc")
                nc.vector.tensor_mul(
                    out=tmp[:],
                    in0=ps[:, :, :HD],
                    in1=recip_sb[:, sl, :].to_broadcast([P, wid, HD]),
                )
                nc.vector.tensor_add(out=dst[:, sl, :], in0=tmp[:], in1=av_sb[:, sl, :])

            for ih in range(2):
                finish_round(ps_p0[ih], slice(ih * 4, ih * 4 + 4), 0, h_a)

            # ---- rounds 1..4 (last round in quarters so the output DMA
            # overlaps the trailing matmuls; separate PSUM tiles per quarter
            # keep Tile's bank tracker from serializing the finish) ----
            for r in range(1, N_ROUNDS):
                h_cur = h_a if r % 2 == 1 else h_b
                h_next = h_b if r % 2 == 1 else h_a
                last = r == N_ROUNDS - 1
                dst = out_sb if last else h_next
                nchunk, wid = (4, 2)
                for ih in range(nchunk):
                    ps_p = psum_small.tile(
                        [P, wid, HD], f32, name=f"ps_p{r}_{ih}", tag="ps_small"
                    )
                    for il in range(wid):
                        it = ih * wid + il
                        for jt in range(NT_S):
                            nc.tensor.matmul(
                                ps_p[:, il, :],
                                wt_sb[:, jt, ts(it, P)],
                                h_cur[:, jt, :],
                                start=(jt == 0),
                                stop=(jt == NT_S - 1),
                            )
                    sl = slice(ih * wid, ih * wid + wid)
                    finish_round(ps_p, sl, r, dst)
                    if last:
                        nc.sync.dma_start(out_t[:, sl, :], out_sb[:, sl, :])

    nc.compile()
    return nc


def _prep_inputs(hidden_states, attention_mask, Wq, bq, Wk, bk, Wv, bv, src, dst):
    bf = ml_dtypes.bfloat16
    hs = np.asarray(hidden_states, np.float32).reshape(S, HIDDEN)
    scale = 1.0 / np.sqrt(HD)

    hsT = hs.T.astype(bf)

    WqT = np.asarray(Wq, np.float32).T * scale  # [HIDDEN, HIDDEN]
    WkT = np.asarray(Wk, np.float32).T
    WvT = np.asarray(Wv, np.float32).T
    # Zero biases per setup_inputs; the kernel folds no bias path, so be loud
    # if that assumption is ever violated rather than silently wrong.
    assert not (np.any(np.asarray(bq)) or np.any(np.asarray(bk)) or np.any(np.asarray(bv))), \
        "nonzero qkv biases are not supported by this kernel"

    # dense adjacency in [src, dst] layout, combined with the attention mask
    ok = (np.asarray(attention_mask, np.float32).reshape(S) > 0)
    adjT = np.zeros((S, S), np.float32)
    adjT[np.asarray(src), np.asarray(dst)] = 1.0
    adjT *= ok[:, None]
    adjT *= ok[None, :]
    adjT = adjT.astype(bf)

    in_maps = []
    for h in range(NH):
        sl = slice(h * HD, (h + 1) * HD)
        wqkv = np.zeros((KDIM, P + HD), np.float32)
        wqkv[:, :HD] = WqT[:, sl]
        wqkv[:, HD:P] = WkT[:, sl]
        wqkv[:, P:] = WvT[:, sl]
        wqkv_packed = (
            wqkv.reshape(NT_K, P, P + HD)
            .transpose(1, 0, 2)
            .reshape(P, NT_K * (P + HD))
        )
        in_maps.append(
            {
                "hsT": hsT,
                "wqkv": wqkv_packed.astype(bf),
                "adjT": adjT,
            }
        )
    return in_maps


def kernel(**inputs):
    from concourse.bass_utils import run_bass_kernel_spmd

    if "nc" not in _CACHED:
        _CACHED["nc"] = _build_module()
    nc = _CACHED["nc"]

    in_maps = _prep_inputs(**inputs)
    import os

    trace = bool(int(os.environ.get("KERNEL_TRACE", "0")))
    res = run_bass_kernel_spmd(
        nc,
        in_maps,
        core_ids=list(range(NH)),
        trace=trace,
        trace_cores=list(range(NH)) if trace else None,
    )
    _CACHED["last_results"] = res

    out = np.concatenate([res.results[h]["out"] for h in range(NH)], axis=1)
    return out.reshape(1, S, NH * HD).astype(np.float32)

